# revision 20
# baseline (speedup 1.0000x reference)
"""Trainium2 Bass kernel for nn_CrossAttentionModel (8 NeuronCores).

Strategy (v3): PURE batch-parallel, zero collectives.

Measured on this fabric: any collective costs ~10us of latency, the
auto-inserted prelude barrier another ~10us, they serialize on the cc
stream, and the first sync point absorbs the full cross-core NEFF start
skew (~35-40us) into the slowest-measured core.  A d-sharded encoder +
ReduceScatter therefore has a ~100us floor even though it moves 3.4x
fewer bytes.  Replicating the (small) encoder weights and giving each
core 16 whole samples runs collective-free: per-core cost is just its
own DMA stream (13.1 MB bf16 ~ 37us at 358 GB/s) overlapped with PE.

Encoder: per core, one contiguous p-major bf16 blob pack[128, 160*320]:
for each of 160 k-tiles (128 d-rows) the 320 cols are [a(s,i) 48 | v 16
| W1^T 128 | W2^T 128].  One DMA per 8 tiles.  Per tile ONE matmul:
stationary act[128,64], moving w12[128,256] -> psum[64,256] accumulates
aud rows (s,i)x cols 0:128 and vis rows 48:64 x cols 128:256 (the two
cross quadrants are garbage and unused).  Bias is seeded by a rank-1
ones x [b1|b2] matmul.

Attention (on-chip, no DRAM round-trip): rows stay in the [(s,i), k]
layout.  Block-diag kron(I16, A^T) stationaries make the 3x3 channel
mixes one matmul per branch-half; per-sample block-diag enc tiles
(bd_a/bd_v, 16 small SBUF DMAs each) turn the per-sample K=3 attention
maps into 4 dense K=48 matmuls per branch-half; tanh on ACT; H and out
projections are dense matmuls with the two branches packed at psum
partition bases 0/32.  The [3,(s,k)] gathers needed for the Wa/Wv terms
and the residual are extracted from bd_a/bd_v with a tile(I3) matmul
(block-diag zeros kill the cross-sample terms), because strided-
partition DMA *sources* are silently broken on this stack.
"""
import sys
sys.path.insert(0, "/opt/trn_rl_repo")

import numpy as np
import concourse.bass as bass
import concourse.mybir as mybir
import concourse.tile as tile
from concourse import bacc
from concourse.bass_utils import run_bass_kernel_spmd

F32 = mybir.dt.float32
BF16 = mybir.dt.bfloat16
AF = mybir.ActivationFunctionType

# ---- problem constants (hardcoded; kernel.py must be self-contained) ----
B, C, H, W = 128, 3, 512, 640
D = 20480            # (H//4) * (W//4)
DE = 128             # encoder dim
DA = 32              # attention dim
NC_ = 8              # cores
SL = B // NC_        # 16 samples per core
SK = SL * DE         # 2048 = (sample, enc) flattened cols
NT = D // 128        # 160 k-tiles (full contraction, per core)
TW = 64 + 2 * DE     # 320 cols per k-tile in the stream blob
CH = 8               # k-tiles per stream DMA
NCH = NT // CH       # 20 stream chunks


def _np_dt(dt):
    return mybir.dt.np(dt)


def build_bass():
    nc = bacc.Bacc("TRN2", target_bir_lowering=False, debug=False,
                   num_devices=NC_)

    # ---- per-core DRAM parameters ----
    pack = nc.declare_dram_parameter("pack", [128, NT * TW], BF16,
                                     isOutput=False)
    ones64 = nc.declare_dram_parameter("ones64", [1, 64], BF16,
                                       isOutput=False)
    brow = nc.declare_dram_parameter("brow", [1, 256], BF16, isOutput=False)
    bdAaT = nc.declare_dram_parameter("bdAaT", [48, 48], BF16, isOutput=False)
    bdAvT = nc.declare_dram_parameter("bdAvT", [48, 48], BF16, isOutput=False)
    tI3 = nc.declare_dram_parameter("tI3", [48, 3], BF16, isOutput=False)
    wa3 = nc.declare_dram_parameter("wa3", [3, DA], BF16, isOutput=False)
    wv3 = nc.declare_dram_parameter("wv3", [3, DA], BF16, isOutput=False)
    wcaT = nc.declare_dram_parameter("wcaT", [2 * DE, DA], BF16,
                                     isOutput=False)
    wcvT = nc.declare_dram_parameter("wcvT", [2 * DE, DA], BF16,
                                     isOutput=False)
    wh6 = nc.declare_dram_parameter("wh6", [2 * DA, 6], BF16, isOutput=False)
    out = nc.declare_dram_parameter("out", [SL, 3, 2 * DE], F32,
                                    isOutput=True)

    bdst_a = nc.dram_tensor("bdst_a", [48, SK], BF16)
    bdst_v = nc.dram_tensor("bdst_v", [48, SK], BF16)

    with tile.TileContext(nc) as tc:
        with (
            tc.tile_pool(name="consts", bufs=1) as cpool,
            tc.tile_pool(name="stream", bufs=3) as spool,
            tc.tile_pool(name="sb", bufs=1) as sb,
        ):
            # ---------- small consts (gpsimd queue) ----------
            ones_t = cpool.tile([1, 64], BF16)
            nc.sync.dma_start(ones_t[:], ones64[:])
            brow_t = cpool.tile([1, 256], BF16)
            nc.sync.dma_start(brow_t[:], brow[:])
            bdAa_t = cpool.tile([48, 48], BF16)
            nc.scalar.dma_start(bdAa_t[:], bdAaT[:])
            bdAv_t = cpool.tile([48, 48], BF16)
            nc.scalar.dma_start(bdAv_t[:], bdAvT[:])
            tI3_t = cpool.tile([48, 3], BF16)
            nc.scalar.dma_start(tI3_t[:], tI3[:])
            wa3_t = cpool.tile([3, DA], BF16)
            nc.scalar.dma_start(wa3_t[:], wa3[:])
            wv3_t = cpool.tile([3, DA], BF16)
            nc.scalar.dma_start(wv3_t[:], wv3[:])
            wca_lo = cpool.tile([DE, DA], BF16)
            nc.scalar.dma_start(wca_lo[:], wcaT[0:DE, :])
            wca_hi = cpool.tile([DE, DA], BF16)
            nc.scalar.dma_start(wca_hi[:], wcaT[DE:2 * DE, :])
            wcv_lo = cpool.tile([DE, DA], BF16)
            nc.scalar.dma_start(wcv_lo[:], wcvT[0:DE, :])
            wcv_hi = cpool.tile([DE, DA], BF16)
            nc.scalar.dma_start(wcv_hi[:], wcvT[DE:2 * DE, :])
            wh6_t = cpool.tile([2 * DA, 6], BF16)
            nc.scalar.dma_start(wh6_t[:], wh6[:])

            # block-diag enc tiles, zeroed early (runs during the encoder)
            bd_a = sb.tile([48, SK], BF16, tag="bd_a")
            nc.vector.memset(bd_a[:], 0.0)
            bd_v = sb.tile([48, SK], BF16, tag="bd_v")
            nc.vector.memset(bd_v[:], 0.0)
            # zeroed DRAM staging for the diagonal scatter (DRAM APs are
            # byte-linear, so a diagonal dst is legal there; SBUF partition
            # strides are not) -- zero-writes run during the encoder
            nc.scalar.dma_start(bdst_a[:], bd_a[:])
            nc.scalar.dma_start(bdst_v[:], bd_v[:])

            # ---------- phase 1: encoder (all 160 k-tiles, one psum) -----
            enc_sb = sb.tile([64, 256], BF16, tag="enc_sb")
            with tc.tile_pool(name="enc_ps", bufs=1, space="PSUM") as eps:
                psum = eps.tile([64, 256], F32, tag="enc")
                nc.tensor.matmul(psum[:], ones_t[:], brow_t[:],
                                 start=True, stop=False)
                for ch in range(NCH):
                    st = spool.tile([128, CH * TW], BF16, tag="st")
                    qeng = nc.sync if ch % 2 == 0 else nc.gpsimd
                    qeng.dma_start(
                        st[:], pack[:, ch * CH * TW:(ch + 1) * CH * TW])
                    for j in range(CH):
                        o = j * TW
                        last = ch == NCH - 1 and j == CH - 1
                        nc.tensor.matmul(psum[:], st[:, o:o + 64],
                                         st[:, o + 64:o + TW],
                                         start=False, stop=last)
                nc.vector.tensor_copy(enc_sb[:], psum[:])

            # ---------- phase 2: attention (on-chip, 16 samples) ---------
            # vis replicated x3 into (s,i) rows  (dst-strided DMA is fine)
            visrep = sb.tile([48, 128], BF16, tag="visrep")
            v3v = visrep[:].rearrange("(s i) k -> s i k", i=3)
            for i in range(3):
                nc.scalar.dma_start(v3v[:, i, :], enc_sb[48:64, 128:256])

            # diagonal scatter via DRAM: one write with a diagonal byte-AP
            # per branch, then one contiguous read-back into the zeroed tile
            from concourse.ap import AP as _AP
            diag_a = _AP(bdst_a, 0, [[3 * SK + DE, SL], [SK, 3], [1, DE]])
            diag_v = _AP(bdst_v, 0, [[3 * SK + DE, SL], [SK, 3], [1, DE]])
            nc.sync.dma_start(diag_a, enc_sb[0:48, 0:128])
            nc.scalar.dma_start(diag_v, visrep[:])
            nc.sync.dma_start(bd_a[:], bdst_a[:])
            nc.scalar.dma_start(bd_v[:], bdst_v[:])

            # B = A @ av via block-diag kron(I16, A^T):  b48 [48, 512] =
            # [Aa@aud | Aa@vis | Av@aud | Av@vis] in m-halves of 128
            b48 = sb.tile([48, 512], BF16, tag="b48")
            av6 = sb.tile([6, SK], BF16, tag="av6")
            av6a = sb.tile([3, SK], BF16, tag="av6a")
            av6v = sb.tile([3, SK], BF16, tag="av6v")
            with (
                tc.tile_pool(name="bp_ps", bufs=1, space="PSUM") as bps,
                tc.tile_pool(name="g3_ps", bufs=2, space="PSUM") as gps,
            ):
                pb48 = bps.tile([48, 512], F32, tag="pb48")
                aud_rhs = enc_sb[0:48, 0:128]
                nc.tensor.matmul(pb48[:, 0:128], bdAa_t[:], aud_rhs,
                                 start=True, stop=True)
                nc.tensor.matmul(pb48[:, 128:256], bdAa_t[:], visrep[:],
                                 start=True, stop=True)
                nc.tensor.matmul(pb48[:, 256:384], bdAv_t[:], aud_rhs,
                                 start=True, stop=True)
                nc.tensor.matmul(pb48[:, 384:512], bdAv_t[:], visrep[:],
                                 start=True, stop=True)
                nc.vector.tensor_copy(b48[:], pb48[:])

                # [3,(s,k)] gathers via tile(I3) against the block-diags
                for q in range(4):
                    ck = slice(q * 512, (q + 1) * 512)
                    ga = gps.tile([3, 512], F32, tag="ga")
                    nc.tensor.matmul(ga[:], tI3_t[:], bd_a[:, ck],
                                     start=True, stop=True)
                    nc.vector.tensor_copy(av6a[:, ck], ga[:])
                    gv = gps.tile([3, 512], F32, tag="gv")
                    nc.tensor.matmul(gv[:], tI3_t[:], bd_v[:, ck],
                                     start=True, stop=True)
                    nc.vector.tensor_copy(av6v[:, ck], gv[:])
            nc.gpsimd.dma_start(av6[0:3, :], av6a[:])
            nc.gpsimd.dma_start(av6[3:6, :], av6v[:])

            with (
                tc.tile_pool(name="att_ps", bufs=2, space="PSUM") as aps,
                tc.tile_pool(name="h_ps", bufs=2, space="PSUM") as hps,
                tc.tile_pool(name="o_ps", bufs=2, space="PSUM") as ops_,
            ):
                # ---------- att = tanh((enc^T @ B) / 16) ----------
                att = {}
                bd = {"a": bd_a, "v": bd_v}
                for bi, br in enumerate(("a", "v")):
                    for half in (0, 1):
                        lhs_off = bi * 256 + half * 128
                        att_sb = sb.tile([128, SK], BF16,
                                         tag=f"att_{br}_{half}",
                                         name=f"att_{br}_{half}")
                        att[(br, half)] = att_sb
                        for q in range(4):
                            ck = slice(q * 512, (q + 1) * 512)
                            pa = aps.tile([128, 512], F32, tag="attp")
                            nc.tensor.matmul(
                                pa[:], b48[:, lhs_off:lhs_off + 128],
                                bd[br][:, ck], start=True, stop=True)
                            nc.scalar.activation(
                                att_sb[:, ck], pa[:], AF.Tanh, scale=0.0625)

                # ---------- H = relu(att @ Wc^T + enc^T @ W) ----------
                # audio rows 0:32, visual rows 32:64 of one packed psum
                ht = sb.tile([2 * DA, SK], BF16, tag="ht")
                for q in range(4):
                    ck = slice(q * 512, (q + 1) * 512)
                    ph = hps.tile([2 * DA, 512], F32, tag="ph")
                    nc.tensor.matmul(ph[0:DA, :], wa3_t[:], av6a[:, ck],
                                     start=True, stop=False)
                    nc.tensor.matmul(ph[0:DA, :], wca_lo[:],
                                     att[("a", 0)][:, ck],
                                     start=False, stop=False)
                    nc.tensor.matmul(ph[0:DA, :], wca_hi[:],
                                     att[("a", 1)][:, ck],
                                     start=False, stop=True)
                    nc.tensor.matmul(ph[DA:2 * DA, :], wv3_t[:],
                                     av6v[:, ck], start=True, stop=False)
                    nc.tensor.matmul(ph[DA:2 * DA, :], wcv_lo[:],
                                     att[("v", 0)][:, ck],
                                     start=False, stop=False)
                    nc.tensor.matmul(ph[DA:2 * DA, :], wcv_hi[:],
                                     att[("v", 1)][:, ck],
                                     start=False, stop=True)
                    nc.vector.tensor_relu(ht[:, ck], ph[:])

                # ---------- out = Wh @ H + enc ----------
                outsb = sb.tile([6, SK], F32, tag="outsb")
                for q in range(4):
                    ck = slice(q * 512, (q + 1) * 512)
                    po = ops_.tile([6, 512], F32, tag="po")
                    nc.tensor.matmul(po[:], wh6_t[:], ht[:, ck],
                                     start=True, stop=True)
                    nc.vector.tensor_add(outsb[:, ck], po[:], av6[:, ck])

            nc.gpsimd.dma_start(
                out[:, :, 0:DE].transpose([1, 0, 2]),
                outsb[0:3, :].rearrange("c (s k) -> c s k", k=DE))
            nc.gpsimd.dma_start(
                out[:, :, DE:2 * DE].transpose([1, 0, 2]),
                outsb[3:6, :].rearrange("c (s k) -> c s k", k=DE))

    nc.compile()
    return nc


_NC_CACHE = None


def _get_nc():
    global _NC_CACHE
    if _NC_CACHE is None:
        _NC_CACHE = build_bass()
    return _NC_CACHE


def _prep_inputs(f1_norm, f2_norm, W1, b1, W2, b2, Aa, Av, Wa, Wv,
                 Wca, Wcv, Wha, Whv):
    f1_norm = np.asarray(f1_norm, dtype=np.float32)
    f2_norm = np.asarray(f2_norm, dtype=np.float32)
    bf = _np_dt(BF16)

    a_ds = f1_norm[:, :, ::4, ::4].reshape(B, 3, D)        # (B, 3, D)
    v_ds = f2_norm[:, ::4, ::4].reshape(B, D)
    w1T = np.asarray(W1).T.astype(bf)                      # (D, 128)
    w2T = np.asarray(W2).T.astype(bf)

    eye16 = np.eye(16, dtype=np.float32)
    wh6_np = np.zeros((2 * DA, 6), dtype=np.float32)
    wh6_np[0:DA, 0:3] = np.asarray(Wha).T
    wh6_np[DA:2 * DA, 3:6] = np.asarray(Whv).T
    consts = {
        "ones64": np.ones((1, 64), dtype=np.float32).astype(bf),
        "brow": np.concatenate([np.asarray(b1), np.asarray(b2)])[None, :]
        .astype(bf),
        "bdAaT": np.kron(eye16, np.asarray(Aa).T).astype(bf),
        "bdAvT": np.kron(eye16, np.asarray(Av).T).astype(bf),
        "tI3": np.tile(np.eye(3, dtype=np.float32), (16, 1)).astype(bf),
        "wa3": np.ascontiguousarray(np.asarray(Wa).T).astype(bf),
        "wv3": np.ascontiguousarray(np.asarray(Wv).T).astype(bf),
        "wcaT": np.ascontiguousarray(np.asarray(Wca).T).astype(bf),
        "wcvT": np.ascontiguousarray(np.asarray(Wcv).T).astype(bf),
        "wh6": wh6_np.astype(bf),
    }

    in_maps = []
    for i in range(NC_):
        sl = slice(i * SL, (i + 1) * SL)
        # stream[d, 0:48] = a[(s,i)], [48:64] = v[s], then W1^T | W2^T
        aT48 = a_ds[sl].transpose(2, 0, 1).reshape(D, 48).astype(bf)
        vT16 = v_ds[sl].T.astype(bf)
        stream = np.concatenate([aT48, vT16, w1T, w2T], axis=1)  # (D, 320)
        blob = np.ascontiguousarray(
            stream.reshape(NT, 128, TW).transpose(1, 0, 2)
        ).reshape(128, NT * TW)
        m = {"pack": blob}
        m.update(consts)
        in_maps.append(m)
    return in_maps


def _run(inputs, trace=False):
    nc = _get_nc()
    in_maps = _prep_inputs(**inputs)
    res = run_bass_kernel_spmd(nc, in_maps, list(range(NC_)), trace=trace)
    full = np.concatenate([res.results[i]["out"] for i in range(NC_)], axis=0)
    return full.astype(np.float32, copy=False), res


def kernel(**inputs):
    out, _ = _run(inputs, trace=False)
    return out
